# revision 37
# baseline (speedup 1.0000x reference)
"""Distributed GIN (3-layer) kernel for 8 TRN2 NeuronCores.

Sharding: nodes (and their incident in-edges) are partitioned across the 8
cores; each core keeps its node-feature shard resident in SBUF (transposed,
bf16).  Between layers the per-core shards are AllGathered into a bf16
"gather table" in DRAM; neighbor aggregation is a dma_gather (256B/edge) +
one-hot matmul segment-sum on the tensor engine.  MLP weights are replicated
(bf16, fp32 PSUM accumulate).  Per-graph pooled sums are computed per-core on
the PE and reduced on the host.

Perf notes (8.14ms baseline -> 2.40ms):
  - dma_gather descriptor generation runs on one Q7 core-pair per swdge
    queue; issuing the 4 per-quadrant gathers of each super-round on queues
    0-3 pipelines desc-gen ~4x (the former 7.1ms wall).
  - scatter one-hots and pooling one-hots are precomputed on the host and
    DMA'd in (DVE is_equal on HW ran at ~7ns/elem and was a 5.4ms wall).
  - node-to-block positions are balanced per core (greedy 4-dim bin packing
    with 512/640 two-tier caps) so the shared gather-slot capacities carry
    only ~2% padding instead of 25%.
  - h / MLP pipeline in bf16 halves SBUF and PE cost and frees room for a
    12-deep gather ring (covers desc-gen + transfer latency).
"""

import ml_dtypes
import numpy as np

N_NODES = 100000
N_EDGES = 1600000
F = 128              # feature dim (= hidden dim)
L = 3                # GIN layers
G = 64               # graphs
NC = 8               # NeuronCores
PER = N_NODES // NC  # 12500 real nodes per core
BLK = 128
NBLK = (PER + BLK - 1) // BLK          # 98 node blocks per core
PERP = NBLK * BLK                      # 12544 padded rows per core
TROWS = NC * PERP                      # 100352 table rows
NQ = 4
QS = TROWS // NQ                       # 25088 rows per quadrant (< int16 max)
RBLK = 4                               # dst blocks per PSUM round (4 PSUM banks)
NR = (NBLK + RBLK - 1) // RBLK         # 25 rounds
GRBLK = 8                              # dst blocks per gather call (2 rounds)
NGR = (NBLK + GRBLK - 1) // GRBLK      # 13 gather super-rounds
MLP_CHUNK = 512
REPS = 1

_F32 = np.float32
_BF16 = ml_dtypes.bfloat16





def _round_blocks(r):
    b0 = r * RBLK
    return range(b0, min(b0 + RBLK, NBLK))


def _gr_blocks(g):
    b0 = g * GRBLK
    return range(b0, min(b0 + GRBLK, NBLK))


def _balance_positions(dvec):
    """Assign a core's PER nodes to NBLK blocks of <=128 nodes, keeping the
    per-(block, src-quadrant) in-edge loads under two-tier caps so the shared
    capacities K land at 4 (blocks 0..93) / 5 (blocks 94..97).

    dvec: [PER, NQ] per-node in-degree by src quadrant.
    Returns assign: [PER] block index per node.
    """
    NT2 = 4
    capv = np.concatenate([
        np.full(NBLK - NT2, 4 * BLK), np.full(NT2, 5 * BLK)
    ])
    order = np.argsort(-dvec.sum(1), kind="stable")
    loads = np.zeros((NBLK, NQ), dtype=np.int64)
    space = np.full(NBLK, BLK, dtype=np.int64)
    assign = np.zeros(PER, dtype=np.int64)
    for i in order:
        di = dvec[i]
        cand = loads + di
        ok = (cand <= capv[:, None]).all(1) & (space > 0)
        score = (cand.max(1) - capv).astype(np.float64)
        if ok.any():
            score[~ok] = 1e18
        else:
            score[space <= 0] = 1e18
        b = int(np.argmin(score))
        assign[i] = b
        loads[b] += di
        space[b] -= 1
    return assign


def _build_structures(edge_index, batch):
    """Host-side preprocessing: per-core padded edge-slot streams.

    Returns a dict with per-core planes plus the shared capacity map.
    """
    src = np.asarray(edge_index[0], dtype=np.int64)
    dst = np.asarray(edge_index[1], dtype=np.int64)

    # node -> core stays contiguous (so src quadrant = core//2 is fixed);
    # positions within a core are balanced so per-(q, block) edge counts are
    # nearly equal across cores (shared K ~= demand, ~2% slot padding).
    q_of_edge = (src // PER) // 2
    dvec_all = np.zeros((N_NODES, NQ), dtype=np.int64)
    np.add.at(dvec_all, (dst, q_of_edge), 1)

    pos_of = np.zeros(N_NODES, dtype=np.int64)       # node -> 0..PERP-1
    perm = np.full((NC, PERP), -1, dtype=np.int64)   # (core, pos) -> node
    for c in range(NC):
        nodes = np.arange(c * PER, (c + 1) * PER)
        assign = _balance_positions(dvec_all[nodes])
        posctr = np.zeros(NBLK, dtype=np.int64)
        for i in range(PER):
            b = assign[i]
            p = b * BLK + posctr[b]
            posctr[b] += 1
            pos_of[nodes[i]] = p
            perm[c, p] = nodes[i]

    srow = PERP * (src // PER) + pos_of[src]
    qloc = (srow % QS).astype(np.int16)

    core_of_edge = dst // PER
    dst_local = pos_of[dst]
    b_of_edge = dst_local // BLK

    # per-core (q, b) counts -> shared capacities (in 128-edge tiles)
    counts = np.zeros((NC, NQ, NBLK), dtype=np.int64)
    np.add.at(counts, (core_of_edge, q_of_edge, b_of_edge), 1)
    K = np.ceil(counts.max(axis=0) / BLK).astype(np.int64)  # [NQ, NBLK]

    # static slot-stream layout: gather super-rounds -> q -> blocks
    group_off = np.zeros((NQ, NBLK), dtype=np.int64)
    tile_of = {}      # (q, b) -> global tile index of the block's first tile
    call_meta = []    # per (g, q): (slot_offset, n_slots)
    off = 0
    for g in range(NGR):
        for q in range(NQ):
            call_off = off
            for b in _gr_blocks(g):
                group_off[q, b] = off
                tile_of[(q, b)] = off // BLK
                off += int(K[q, b]) * BLK
            call_meta.append((g, q, call_off, off - call_off))
    tot_slots = off
    assert tot_slots % BLK == 0
    tot_tiles = tot_slots // BLK

    # one-hot tile layout: ordered by (round r, q, b in round, t); per (r, q)
    # a contiguous run of oh tiles is DMA'd per round.
    oh_tile_pos = {}  # (q, b, t) -> position in oh plane (tile units)
    oh_off = {}       # (r, q) -> (tile_pos_start, n_tiles)
    opos = 0
    for r in range(NR):
        for q in range(NQ):
            start = opos
            for b in _round_blocks(r):
                for t in range(int(K[q, b])):
                    oh_tile_pos[(q, b, t)] = opos
                    opos += 1
            oh_off[(r, q)] = (start, opos - start)
    assert opos == tot_tiles

    idx_planes = []
    oh_planes = []
    ohg_planes = []
    jj = np.arange(BLK, dtype=np.int64)
    for c in range(NC):
        sel = core_of_edge == c
        eq = q_of_edge[sel]
        eb = b_of_edge[sel]
        eloc = qloc[sel]
        edrel = (dst_local[sel] - eb * BLK).astype(np.int64)

        idx = np.zeros(tot_slots, dtype=np.int16)
        drel = np.full(tot_slots, -1, dtype=np.int64)
        # order edges by (q, b); place each (q, b) run at its static offset
        order = np.lexsort((eb, eq))
        eq, eb, eloc, edrel = eq[order], eb[order], eloc[order], edrel[order]
        gid = eq * NBLK + eb
        run_starts = np.zeros(NQ * NBLK + 1, dtype=np.int64)
        np.add.at(run_starts, gid + 1, 1)
        run_starts = np.cumsum(run_starts)
        flat_off = group_off.reshape(-1)
        pos = flat_off[gid] + (np.arange(gid.size) - run_starts[gid])
        idx[pos] = eloc
        drel[pos] = edrel

        idx_plane = np.tile(idx.reshape(-1, 16).T, (8, 1))          # [128, tot/16]
        idx_planes.append(np.ascontiguousarray(idx_plane))

        # scatter one-hots, slot-stream tile order -> oh plane order
        drel_t = drel.reshape(tot_tiles, BLK)                       # [tile, e]
        oh = (drel_t[:, :, None] == jj[None, None, :])              # [tile, e, j]
        ohp = np.zeros((BLK, tot_tiles, BLK), dtype=_BF16)          # [e, opos, j]
        src_tiles = np.empty(tot_tiles, dtype=np.int64)
        for q in range(NQ):
            for b in range(NBLK):
                bt = tile_of[(q, b)]
                for t in range(int(K[q, b])):
                    src_tiles[oh_tile_pos[(q, b, t)]] = bt + t
        ohp[:, :, :] = oh[src_tiles].transpose(1, 0, 2).astype(_BF16)
        oh_planes.append(np.ascontiguousarray(ohp.reshape(BLK, tot_tiles * BLK)))

        bgfull = np.full(NBLK * BLK, -1, dtype=np.int64)
        occ = perm[c] >= 0
        bgfull[occ] = np.asarray(batch, dtype=np.int64)[perm[c][occ]]
        bgt = bgfull.reshape(NBLK, BLK)                             # [b, e]
        ohg = (bgt[:, :, None] == np.arange(G, dtype=np.int64)[None, None, :])
        ohg_plane = ohg.transpose(1, 0, 2).astype(_BF16)            # [e, b, g]
        ohg_planes.append(np.ascontiguousarray(ohg_plane.reshape(BLK, NBLK * G)))

    return {
        "K": K,
        "tile_of": tile_of,
        "call_meta": call_meta,
        "oh_off": oh_off,
        "tot_slots": tot_slots,
        "tot_tiles": tot_tiles,
        "idx_planes": idx_planes,
        "oh_planes": oh_planes,
        "ohg_planes": ohg_planes,
        "perm": perm,
    }


def _build_program(struct, eps_vals):
    import concourse.bacc as bacc
    import concourse.mybir as mybir
    from concourse import tile

    dt = mybir.dt
    AF = mybir.ActivationFunctionType
    OP = mybir.AluOpType

    tot_slots = struct["tot_slots"]
    tot_tiles = struct["tot_tiles"]
    call_meta = struct["call_meta"]
    oh_off = struct["oh_off"]

    nc = bacc.Bacc("TRN2", target_bir_lowering=False, num_swdge_queues=4)

    # ---- kernel parameters (per-core values via in_maps) ----
    xT_p = nc.declare_dram_parameter("xT", [F, PERP], dt.bfloat16, isOutput=False)
    idx_p = nc.declare_dram_parameter("idx", [128, tot_slots // 16], dt.int16, isOutput=False)
    ohp_p = nc.declare_dram_parameter("ohp", [128, tot_tiles * BLK], dt.bfloat16, isOutput=False)
    ohg_p = nc.declare_dram_parameter("ohg", [128, NBLK * G], dt.bfloat16, isOutput=False)
    ident_p = nc.declare_dram_parameter("ident", [128, 128], dt.bfloat16, isOutput=False)
    Wp1_p = nc.declare_dram_parameter("Wp1", [F, F], dt.bfloat16, isOutput=False)
    bp1_p = nc.declare_dram_parameter("bp1", [F, 1], dt.float32, isOutput=False)
    Wp2_p = nc.declare_dram_parameter("Wp2", [F, F], dt.bfloat16, isOutput=False)
    bp2_p = nc.declare_dram_parameter("bp2", [F, 1], dt.float32, isOutput=False)
    W1_p = nc.declare_dram_parameter("W1s", [L, F, F], dt.bfloat16, isOutput=False)
    b1_p = nc.declare_dram_parameter("b1s", [L, F, 1], dt.float32, isOutput=False)
    W2_p = nc.declare_dram_parameter("W2s", [L, F, F], dt.bfloat16, isOutput=False)
    b2_p = nc.declare_dram_parameter("b2s", [L, F, 1], dt.float32, isOutput=False)
    out_p = nc.declare_dram_parameter("out", [G, L * F], dt.float32, isOutput=True)

    # ---- internal DRAM ----
    ag_in = nc.dram_tensor("ag_in", [PERP, F], dt.bfloat16)
    tables = [
        nc.dram_tensor(f"table{l}", [TROWS, F], dt.bfloat16, addr_space="Shared")
        for l in range(L)
    ]

    with tile.TileContext(nc) as tc:
        with (
            tc.tile_pool(name="const", bufs=1) as cpool,
            tc.tile_pool(name="ht", bufs=1) as hpool,
            tc.tile_pool(name="gath", bufs=12) as gpool,
            tc.tile_pool(name="idxp", bufs=12) as ipool,
            tc.tile_pool(name="oh", bufs=8) as ohpool,
            tc.tile_pool(name="zz", bufs=2) as zpool,
            tc.tile_pool(name="emit", bufs=4) as epool,
            tc.tile_pool(name="psag", bufs=4, space="PSUM") as psag,
            tc.tile_pool(name="psmlp", bufs=2, space="PSUM") as psmlp,
            tc.tile_pool(name="pstr", bufs=1, space="PSUM") as pstr,
            tc.tile_pool(name="pspool", bufs=1, space="PSUM") as pspool,
        ):
            # ---- load constants / weights ----
            ident_sb = cpool.tile([128, 128], dt.bfloat16, tag="ident")
            nc.sync.dma_start(ident_sb[:], ident_p[:])
            ohg_sb = cpool.tile([128, NBLK, G], dt.bfloat16, tag="ohg")
            nc.sync.dma_start(ohg_sb[:].rearrange("p a b -> p (a b)"), ohg_p[:])

            def _load_w(tag, pslice):
                t = cpool.tile([F, F], dt.bfloat16, tag=tag)
                nc.sync.dma_start(t[:], pslice)
                return t

            def _load_b(tag, pslice):
                t = cpool.tile([F, 1], dt.float32, tag=tag)
                nc.sync.dma_start(t[:], pslice)
                return t

            Wp1 = _load_w("Wp1", Wp1_p[:])
            Wp2 = _load_w("Wp2", Wp2_p[:])
            bp1 = _load_b("bp1", bp1_p[:])
            bp2 = _load_b("bp2", bp2_p[:])
            W1 = [_load_w(f"W1_{l}", W1_p[l][:]) for l in range(L)]
            W2 = [_load_w(f"W2_{l}", W2_p[l][:]) for l in range(L)]
            b1 = [_load_b(f"b1_{l}", b1_p[l][:]) for l in range(L)]
            b2 = [_load_b(f"b2_{l}", b2_p[l][:]) for l in range(L)]

            hT = hpool.tile([F, PERP], dt.bfloat16, tag="hT")
            qsems = [nc.alloc_semaphore(f"gatherq{q}") for q in range(NQ)]

            for _rep in range(REPS):
                pool_psums = []

                def _emit_block(b, layer_out):
                    """Cast+transpose block b of hT; DMA to ag_in (if a table is
                    still needed) and accumulate pooling (if layer_out >= 1)."""
                    ptr = pstr.tile([128, 128], dt.bfloat16, tag="tr")
                    nc.tensor.transpose(ptr[:], hT[:, b * BLK:(b + 1) * BLK], ident_sb[:])
                    hrow = epool.tile([128, 128], dt.bfloat16, tag="hrow")
                    nc.scalar.activation(hrow[:], ptr[:], AF.Copy)
                    if layer_out < L:
                        nc.sync.dma_start(ag_in[b * BLK:(b + 1) * BLK, :], hrow[:])
                    if layer_out >= 1:
                        nc.tensor.matmul(
                            pool_psums[layer_out - 1][:],
                            ohg_sb[:, b, :],
                            hrow[:],
                            start=(b == 0),
                            stop=(b == NBLK - 1),
                            skip_group_check=True,
                        )

                # ---- pre-MLP: hT = relu(relu(x Wp1 + bp1) Wp2 + bp2), transposed;
                # h0 blocks are emitted to ag_in as soon as each chunk lands ----
                o = 0
                while o < PERP:
                    cw = min(MLP_CHUNK, PERP - o)
                    xc = zpool.tile([F, MLP_CHUNK], dt.bfloat16, tag="xc")
                    nc.sync.dma_start(xc[:, :cw], xT_p[:, o:o + cw])
                    p1 = psmlp.tile([F, MLP_CHUNK], dt.float32, tag="mlp")
                    nc.tensor.matmul(p1[:, :cw], Wp1[:], xc[:, :cw])
                    t1 = zpool.tile([F, MLP_CHUNK], dt.bfloat16, tag="t1")
                    nc.scalar.activation(t1[:, :cw], p1[:, :cw], AF.Relu, bias=bp1[:])
                    p2 = psmlp.tile([F, MLP_CHUNK], dt.float32, tag="mlp")
                    nc.tensor.matmul(p2[:, :cw], Wp2[:], t1[:, :cw])
                    nc.scalar.activation(hT[:, o:o + cw], p2[:, :cw], AF.Relu, bias=bp2[:])
                    for k in range(cw // BLK):
                        _emit_block(o // BLK + k, 0)
                    o += cw

                nc.gpsimd.collective_compute(
                    "AllGather", OP.bypass,
                    replica_groups=[list(range(NC))],
                    ins=[ag_in[:]], outs=[tables[0][:]],
                )

                # ---- GIN layers ----
                for l in range(L):
                    pool_psums.append(pspool.tile([G, F], dt.float32, tag="pool", name=f"poolp{l}"))
                    # prescale: hT *= (1 + eps_l)   (table_l already captured h_l)
                    nc.vector.tensor_scalar(
                        hT[:], hT[:], float(1.0 + eps_vals[l]), None, op0=OP.mult
                    )

                    # PSUM accumulation groups are bank-granular: each block gets
                    # its own [F, BLK] psum tile (padded to one bank) and all of
                    # its matmuls are consecutive.  Gathers stay q-major per round
                    # (big calls, queue q -> its own Q7 pair); matmuls consume the
                    # SBUF buffers block-major.
                    K = struct["K"]

                    def _mlp_chunk(o, cw, agg_of):
                        z = zpool.tile([F, MLP_CHUNK], dt.bfloat16, tag="z",
                                       name=f"z_l{l}_o{o}")
                        for k in range(cw // BLK):
                            b = o // BLK + k
                            nc.vector.tensor_tensor(
                                z[:, k * BLK:(k + 1) * BLK],
                                agg_of[b][:],
                                hT[:, b * BLK:(b + 1) * BLK],
                                OP.add,
                            )
                        p1 = psmlp.tile([F, MLP_CHUNK], dt.float32, tag="mlp",
                                        name=f"p1_l{l}_o{o}")
                        nc.tensor.matmul(p1[:, :cw], W1[l][:], z[:, :cw])
                        t1 = zpool.tile([F, MLP_CHUNK], dt.bfloat16, tag="t1",
                                        name=f"t1_l{l}_o{o}")
                        nc.scalar.activation(t1[:, :cw], p1[:, :cw], AF.Relu, bias=b1[l][:])
                        p2 = psmlp.tile([F, MLP_CHUNK], dt.float32, tag="mlp",
                                        name=f"p2_l{l}_o{o}")
                        nc.tensor.matmul(p2[:, :cw], W2[l][:], t1[:, :cw])
                        nc.vector.tensor_scalar(
                            hT[:, o:o + cw], p2[:, :cw], b2[l][:], None, op0=OP.add
                        )
                        for k in range(cw // BLK):
                            _emit_block(o // BLK + k, l + 1)

                    tile_of = struct["tile_of"]
                    vis_done = {b: 0 for b in range(NBLK)}

                    def _issue_gather(g, q, call_off, n_slots, prep):
                        T = n_slots // BLK
                        idxs = ipool.tile([128, n_slots // 16], dt.int16,
                                          tag="idxs", name=f"idxs_l{l}_g{g}_q{q}")
                        nc.sync.dma_start(
                            idxs[:], idx_p[:, call_off // 16:(call_off + n_slots) // 16]
                        )
                        gt = gpool.tile([128, T, 128], dt.bfloat16, tag="gt",
                                        name=f"gt_l{l}_g{g}_q{q}")
                        kw = dict(prepare_only=True, sem=qsems[q]) if prep else {}
                        nc.gpsimd.dma_gather(
                            gt[:],
                            tables[l][q * QS:(q + 1) * QS, :],
                            idxs[:],
                            n_slots,
                            n_slots,
                            F,
                            single_packet=False,
                            queue_num=q,
                            **kw,
                        )
                        return gt, call_off // BLK

                    # g=0 as prepare_only: desc-gen runs on the Q7 pairs while
                    # the table-l AllGather is still in flight; the triggers
                    # fire the transfers the moment the collective lands.
                    gts0 = {}
                    for (gg, q, call_off, n_slots) in call_meta:
                        if gg == 0 and n_slots > 0:
                            gts0[q] = _issue_gather(0, q, call_off, n_slots, False)

                    for g in range(NGR):
                        # issue big gathers (one per quadrant, spanning GRBLK
                        # blocks); queue q -> Q7 pair q so desc-gen pipelines
                        if g == 0:
                            gts = gts0
                        else:
                            gts = {}
                            for (gg, q, call_off, n_slots) in call_meta:
                                if gg == g and n_slots > 0:
                                    gts[q] = _issue_gather(g, q, call_off, n_slots, False)

                        for r in range(g * GRBLK // RBLK,
                                       min((g + 1) * GRBLK, NBLK + RBLK - 1) // RBLK):
                            rblocks = [b for b in _round_blocks(r) if b < NBLK]
                            if not rblocks:
                                continue
                            # load this round's scatter one-hots, per quadrant
                            ohs = {}
                            for q in range(NQ):
                                opos, Tr = oh_off[(r, q)]
                                if Tr == 0 or q not in gts:
                                    continue
                                oh = ohpool.tile([128, Tr, 128], dt.bfloat16, tag="oh",
                                                 name=f"oh_l{l}_r{r}_q{q}")
                                nc.sync.dma_start(
                                    oh[:].rearrange("p a b -> p (a b)"),
                                    ohp_p[:, opos * BLK:(opos + Tr) * BLK],
                                )
                                ohs[q] = (oh, opos)
                            # per-block PSUM accumulators, one full bank each
                            agg_of = {}
                            for b in rblocks:
                                agg_of[b] = psag.tile([F, BLK], dt.float32, tag="agg",
                                                      name=f"agg_l{l}_b{b}")
                                if int(K[:, b].sum()) == 0:
                                    nc.vector.memset(agg_of[b][:], 0.0)
                            for q in range(NQ):
                                if q not in ohs:
                                    continue
                                oh, opos = ohs[q]
                                gt, c0 = gts[q]
                                ot = 0
                                for b in rblocks:
                                    nvis = int(K[:, b].sum())
                                    bt = tile_of[(q, b)]
                                    for t in range(int(K[q, b])):
                                        nc.tensor.matmul(
                                            agg_of[b][:],
                                            gt[:, bt - c0 + t, :],
                                            oh[:, ot + t, :],
                                            start=(vis_done[b] == 0),
                                            stop=(vis_done[b] == nvis - 1),
                                            skip_group_check=True,
                                        )
                                        vis_done[b] += 1
                                    ot += int(K[q, b])
                            # close the round: z, MLP, emit (one chunk per round)
                            o = rblocks[0] * BLK
                            _mlp_chunk(o, (rblocks[-1] + 1) * BLK - o, agg_of)

                    if l + 1 < L:
                        nc.gpsimd.collective_compute(
                            "AllGather", OP.bypass,
                            replica_groups=[list(range(NC))],
                            ins=[ag_in[:]], outs=[tables[l + 1][:]],
                        )
                    # extract pooled sums for this layer
                    pooled_sb = epool.tile([G, F], dt.float32, tag="pooled")
                    nc.scalar.activation(pooled_sb[:], pool_psums[l][:], AF.Copy)
                    nc.sync.dma_start(out_p[:, l * F:(l + 1) * F], pooled_sb[:])

    nc.compile()
    return nc


def _make_in_maps(struct, inputs):
    x = np.asarray(inputs["x"], dtype=_F32)
    ident = np.eye(128, dtype=_F32).astype(_BF16)

    shared = {
        "ident": np.ascontiguousarray(ident),
        "Wp1": np.asarray(inputs["W_pre1"], dtype=_F32).astype(_BF16),
        "bp1": np.asarray(inputs["b_pre1"], dtype=_F32).reshape(F, 1),
        "Wp2": np.asarray(inputs["W_pre2"], dtype=_F32).astype(_BF16),
        "bp2": np.asarray(inputs["b_pre2"], dtype=_F32).reshape(F, 1),
        "W1s": np.asarray(inputs["W1s"], dtype=_F32).astype(_BF16),
        "b1s": np.asarray(inputs["b1s"], dtype=_F32).reshape(L, F, 1),
        "W2s": np.asarray(inputs["W2s"], dtype=_F32).astype(_BF16),
        "b2s": np.asarray(inputs["b2s"], dtype=_F32).reshape(L, F, 1),
    }

    perm = struct["perm"]
    in_maps = []
    for c in range(NC):
        xs = np.zeros((F, PERP), dtype=_F32)
        occ = perm[c] >= 0
        xs[:, occ] = x[perm[c][occ]].T
        xs = xs.astype(_BF16)
        m = dict(shared)
        m["xT"] = xs
        m["idx"] = struct["idx_planes"][c]
        m["ohp"] = struct["oh_planes"][c]
        m["ohg"] = struct["ohg_planes"][c]
        in_maps.append(m)
    return in_maps


def kernel(**inputs):
    from concourse.bass_utils import run_bass_kernel_spmd

    edge_index = np.asarray(inputs["edge_index"])
    batch = np.asarray(inputs["batch"])
    eps = np.asarray(inputs["eps"], dtype=_F32)

    struct = _build_structures(edge_index, batch)
    nc = _build_program(struct, [float(e) for e in eps])
    in_maps = _make_in_maps(struct, inputs)

    res = run_bass_kernel_spmd(nc, in_maps, core_ids=list(range(NC)))
    out = np.zeros((G, L * F), dtype=_F32)
    for c in range(NC):
        out += res.results[c]["out"]
    return out


# revision 39
# speedup vs baseline: 1.0371x; 1.0371x over previous
"""Distributed GIN (3-layer) kernel for 8 TRN2 NeuronCores.

Sharding: nodes (and their incident in-edges) are partitioned across the 8
cores; each core keeps its node-feature shard resident in SBUF (transposed,
bf16).  Between layers the per-core shards are AllGathered into a bf16
"gather table" in DRAM; neighbor aggregation is a dma_gather (256B/edge) +
one-hot matmul segment-sum on the tensor engine.  MLP weights are replicated
(bf16, fp32 PSUM accumulate).  Per-graph pooled sums are computed per-core on
the PE and reduced on the host.

Perf notes (8.14ms baseline -> 2.40ms):
  - dma_gather descriptor generation runs on one Q7 core-pair per swdge
    queue; issuing the 4 per-quadrant gathers of each super-round on queues
    0-3 pipelines desc-gen ~4x (the former 7.1ms wall).
  - scatter one-hots and pooling one-hots are precomputed on the host and
    DMA'd in (DVE is_equal on HW ran at ~7ns/elem and was a 5.4ms wall).
  - node-to-block positions are balanced per core (greedy 4-dim bin packing
    with 512/640 two-tier caps) so the shared gather-slot capacities carry
    only ~2% padding instead of 25%.
  - h / MLP pipeline in bf16 halves SBUF and PE cost and frees room for a
    12-deep gather ring (covers desc-gen + transfer latency).
"""

import ml_dtypes
import numpy as np

N_NODES = 100000
N_EDGES = 1600000
F = 128              # feature dim (= hidden dim)
L = 3                # GIN layers
G = 64               # graphs
NC = 8               # NeuronCores
PER = N_NODES // NC  # 12500 real nodes per core
BLK = 128
NBLK = (PER + BLK - 1) // BLK          # 98 node blocks per core
PERP = NBLK * BLK                      # 12544 padded rows per core
TROWS = NC * PERP                      # 100352 table rows
NQ = 4
QS = TROWS // NQ                       # 25088 rows per quadrant (< int16 max)
RBLK = 4                               # dst blocks per PSUM round (4 PSUM banks)
NR = (NBLK + RBLK - 1) // RBLK         # 25 rounds
GRBLK = 8                              # dst blocks per gather call (2 rounds)
NGR = (NBLK + GRBLK - 1) // GRBLK      # 13 gather super-rounds
MLP_CHUNK = 512
REPS = 1

_F32 = np.float32
_BF16 = ml_dtypes.bfloat16





def _round_blocks(r):
    b0 = r * RBLK
    return range(b0, min(b0 + RBLK, NBLK))


def _gr_blocks(g):
    b0 = g * GRBLK
    return range(b0, min(b0 + GRBLK, NBLK))


def _balance_positions(dvec):
    """Assign a core's PER nodes to NBLK blocks of <=128 nodes, keeping the
    per-(block, src-quadrant) in-edge loads under two-tier caps so the shared
    capacities K land at 4 (blocks 0..93) / 5 (blocks 94..97).

    dvec: [PER, NQ] per-node in-degree by src quadrant.
    Returns assign: [PER] block index per node.
    """
    NT2 = 4
    capv = np.concatenate([
        np.full(NBLK - NT2, 4 * BLK), np.full(NT2, 5 * BLK)
    ])
    order = np.argsort(-dvec.sum(1), kind="stable")
    loads = np.zeros((NBLK, NQ), dtype=np.int64)
    space = np.full(NBLK, BLK, dtype=np.int64)
    assign = np.zeros(PER, dtype=np.int64)
    for i in order:
        di = dvec[i]
        cand = loads + di
        ok = (cand <= capv[:, None]).all(1) & (space > 0)
        score = (cand.max(1) - capv).astype(np.float64)
        if ok.any():
            score[~ok] = 1e18
        else:
            score[space <= 0] = 1e18
        b = int(np.argmin(score))
        assign[i] = b
        loads[b] += di
        space[b] -= 1
    return assign


def _build_structures(edge_index, batch):
    """Host-side preprocessing: per-core padded edge-slot streams.

    Returns a dict with per-core planes plus the shared capacity map.
    """
    src = np.asarray(edge_index[0], dtype=np.int64)
    dst = np.asarray(edge_index[1], dtype=np.int64)

    # node -> core stays contiguous (so src quadrant = core//2 is fixed);
    # positions within a core are balanced so per-(q, block) edge counts are
    # nearly equal across cores (shared K ~= demand, ~2% slot padding).
    q_of_edge = (src // PER) // 2
    dvec_all = np.zeros((N_NODES, NQ), dtype=np.int64)
    np.add.at(dvec_all, (dst, q_of_edge), 1)

    pos_of = np.zeros(N_NODES, dtype=np.int64)       # node -> 0..PERP-1
    perm = np.full((NC, PERP), -1, dtype=np.int64)   # (core, pos) -> node
    for c in range(NC):
        nodes = np.arange(c * PER, (c + 1) * PER)
        assign = _balance_positions(dvec_all[nodes])
        posctr = np.zeros(NBLK, dtype=np.int64)
        for i in range(PER):
            b = assign[i]
            p = b * BLK + posctr[b]
            posctr[b] += 1
            pos_of[nodes[i]] = p
            perm[c, p] = nodes[i]

    srow = PERP * (src // PER) + pos_of[src]
    qloc = (srow % QS).astype(np.int16)

    core_of_edge = dst // PER
    dst_local = pos_of[dst]
    b_of_edge = dst_local // BLK

    # per-core (q, b) counts -> shared capacities (in 128-edge tiles)
    counts = np.zeros((NC, NQ, NBLK), dtype=np.int64)
    np.add.at(counts, (core_of_edge, q_of_edge, b_of_edge), 1)
    K = np.ceil(counts.max(axis=0) / BLK).astype(np.int64)  # [NQ, NBLK]

    # static slot-stream layout: gather super-rounds -> q -> blocks
    group_off = np.zeros((NQ, NBLK), dtype=np.int64)
    tile_of = {}      # (q, b) -> global tile index of the block's first tile
    call_meta = []    # per (g, q): (slot_offset, n_slots)
    off = 0
    for g in range(NGR):
        for q in range(NQ):
            call_off = off
            for b in _gr_blocks(g):
                group_off[q, b] = off
                tile_of[(q, b)] = off // BLK
                off += int(K[q, b]) * BLK
            call_meta.append((g, q, call_off, off - call_off))
    tot_slots = off
    assert tot_slots % BLK == 0
    tot_tiles = tot_slots // BLK

    # one-hot tile layout: ordered by (round r, q, b in round, t); per (r, q)
    # a contiguous run of oh tiles is DMA'd per round.
    oh_tile_pos = {}  # (q, b, t) -> position in oh plane (tile units)
    oh_off = {}       # (r, q) -> (tile_pos_start, n_tiles)
    opos = 0
    for r in range(NR):
        for q in range(NQ):
            start = opos
            for b in _round_blocks(r):
                for t in range(int(K[q, b])):
                    oh_tile_pos[(q, b, t)] = opos
                    opos += 1
            oh_off[(r, q)] = (start, opos - start)
    assert opos == tot_tiles

    idx_planes = []
    oh_planes = []
    ohg_planes = []
    jj = np.arange(BLK, dtype=np.int64)
    for c in range(NC):
        sel = core_of_edge == c
        eq = q_of_edge[sel]
        eb = b_of_edge[sel]
        eloc = qloc[sel]
        edrel = (dst_local[sel] - eb * BLK).astype(np.int64)

        idx = np.zeros(tot_slots, dtype=np.int16)
        drel = np.full(tot_slots, -1, dtype=np.int64)
        # order edges by (q, b); place each (q, b) run at its static offset
        order = np.lexsort((eb, eq))
        eq, eb, eloc, edrel = eq[order], eb[order], eloc[order], edrel[order]
        gid = eq * NBLK + eb
        run_starts = np.zeros(NQ * NBLK + 1, dtype=np.int64)
        np.add.at(run_starts, gid + 1, 1)
        run_starts = np.cumsum(run_starts)
        flat_off = group_off.reshape(-1)
        pos = flat_off[gid] + (np.arange(gid.size) - run_starts[gid])
        idx[pos] = eloc
        drel[pos] = edrel

        idx_plane = np.tile(idx.reshape(-1, 16).T, (8, 1))          # [128, tot/16]
        idx_planes.append(np.ascontiguousarray(idx_plane))

        # scatter one-hots, slot-stream tile order -> oh plane order
        drel_t = drel.reshape(tot_tiles, BLK)                       # [tile, e]
        oh = (drel_t[:, :, None] == jj[None, None, :])              # [tile, e, j]
        ohp = np.zeros((BLK, tot_tiles, BLK), dtype=_BF16)          # [e, opos, j]
        src_tiles = np.empty(tot_tiles, dtype=np.int64)
        for q in range(NQ):
            for b in range(NBLK):
                bt = tile_of[(q, b)]
                for t in range(int(K[q, b])):
                    src_tiles[oh_tile_pos[(q, b, t)]] = bt + t
        ohp[:, :, :] = oh[src_tiles].transpose(1, 0, 2).astype(_BF16)
        oh_planes.append(np.ascontiguousarray(ohp.reshape(BLK, tot_tiles * BLK)))

        bgfull = np.full(NBLK * BLK, -1, dtype=np.int64)
        occ = perm[c] >= 0
        bgfull[occ] = np.asarray(batch, dtype=np.int64)[perm[c][occ]]
        bgt = bgfull.reshape(NBLK, BLK)                             # [b, e]
        ohg = (bgt[:, :, None] == np.arange(G, dtype=np.int64)[None, None, :])
        ohg_plane = ohg.transpose(1, 0, 2).astype(_BF16)            # [e, b, g]
        ohg_planes.append(np.ascontiguousarray(ohg_plane.reshape(BLK, NBLK * G)))

    return {
        "K": K,
        "tile_of": tile_of,
        "call_meta": call_meta,
        "oh_off": oh_off,
        "tot_slots": tot_slots,
        "tot_tiles": tot_tiles,
        "idx_planes": idx_planes,
        "oh_planes": oh_planes,
        "ohg_planes": ohg_planes,
        "perm": perm,
    }


def _build_program(struct, eps_vals):
    import concourse.bacc as bacc
    import concourse.mybir as mybir
    from concourse import tile

    dt = mybir.dt
    AF = mybir.ActivationFunctionType
    OP = mybir.AluOpType

    tot_slots = struct["tot_slots"]
    tot_tiles = struct["tot_tiles"]
    call_meta = struct["call_meta"]
    oh_off = struct["oh_off"]

    nc = bacc.Bacc("TRN2", target_bir_lowering=False, num_swdge_queues=4)

    # ---- kernel parameters (per-core values via in_maps) ----
    xT_p = nc.declare_dram_parameter("xT", [F, PERP], dt.bfloat16, isOutput=False)
    idx_p = nc.declare_dram_parameter("idx", [128, tot_slots // 16], dt.int16, isOutput=False)
    ohp_p = nc.declare_dram_parameter("ohp", [128, tot_tiles * BLK], dt.bfloat16, isOutput=False)
    ohg_p = nc.declare_dram_parameter("ohg", [128, NBLK * G], dt.bfloat16, isOutput=False)
    ident_p = nc.declare_dram_parameter("ident", [128, 128], dt.bfloat16, isOutput=False)
    Wp1_p = nc.declare_dram_parameter("Wp1", [F, F], dt.bfloat16, isOutput=False)
    bp1_p = nc.declare_dram_parameter("bp1", [F, 1], dt.float32, isOutput=False)
    Wp2_p = nc.declare_dram_parameter("Wp2", [F, F], dt.bfloat16, isOutput=False)
    bp2_p = nc.declare_dram_parameter("bp2", [F, 1], dt.float32, isOutput=False)
    W1_p = nc.declare_dram_parameter("W1s", [L, F, F], dt.bfloat16, isOutput=False)
    b1_p = nc.declare_dram_parameter("b1s", [L, F, 1], dt.float32, isOutput=False)
    W2_p = nc.declare_dram_parameter("W2s", [L, F, F], dt.bfloat16, isOutput=False)
    b2_p = nc.declare_dram_parameter("b2s", [L, F, 1], dt.float32, isOutput=False)
    out_p = nc.declare_dram_parameter("out", [G, L * F], dt.float32, isOutput=True)

    # ---- internal DRAM ----
    ag_in = nc.dram_tensor("ag_in", [PERP, F], dt.bfloat16)
    tables = [
        nc.dram_tensor(f"table{l}", [TROWS, F], dt.bfloat16, addr_space="Shared")
        for l in range(L)
    ]

    with tile.TileContext(nc) as tc:
        with (
            tc.tile_pool(name="const", bufs=1) as cpool,
            tc.tile_pool(name="ht", bufs=1) as hpool,
            tc.tile_pool(name="gath", bufs=12) as gpool,
            tc.tile_pool(name="idxp", bufs=12) as ipool,
            tc.tile_pool(name="oh", bufs=8) as ohpool,
            tc.tile_pool(name="zz", bufs=2) as zpool,
            tc.tile_pool(name="emit", bufs=4) as epool,
            tc.tile_pool(name="psag", bufs=4, space="PSUM") as psag,
            tc.tile_pool(name="psmlp", bufs=2, space="PSUM") as psmlp,
            tc.tile_pool(name="pstr", bufs=1, space="PSUM") as pstr,
            tc.tile_pool(name="pspool", bufs=1, space="PSUM") as pspool,
        ):
            # ---- load constants / weights ----
            ident_sb = cpool.tile([128, 128], dt.bfloat16, tag="ident")
            nc.sync.dma_start(ident_sb[:], ident_p[:])
            ohg_sb = cpool.tile([128, NBLK, G], dt.bfloat16, tag="ohg")
            nc.sync.dma_start(ohg_sb[:].rearrange("p a b -> p (a b)"), ohg_p[:])

            def _load_w(tag, pslice):
                t = cpool.tile([F, F], dt.bfloat16, tag=tag)
                nc.sync.dma_start(t[:], pslice)
                return t

            def _load_b(tag, pslice):
                t = cpool.tile([F, 1], dt.float32, tag=tag)
                nc.sync.dma_start(t[:], pslice)
                return t

            Wp1 = _load_w("Wp1", Wp1_p[:])
            Wp2 = _load_w("Wp2", Wp2_p[:])
            bp1 = _load_b("bp1", bp1_p[:])
            bp2 = _load_b("bp2", bp2_p[:])
            W1 = [_load_w(f"W1_{l}", W1_p[l][:]) for l in range(L)]
            W2 = [_load_w(f"W2_{l}", W2_p[l][:]) for l in range(L)]
            b1 = [_load_b(f"b1_{l}", b1_p[l][:]) for l in range(L)]
            b2 = [_load_b(f"b2_{l}", b2_p[l][:]) for l in range(L)]

            hT = hpool.tile([F, PERP], dt.bfloat16, tag="hT")
            qsems = [nc.alloc_semaphore(f"gatherq{q}") for q in range(NQ)]

            for _rep in range(REPS):
                pool_psums = []

                def _emit_block(b, layer_out):
                    """Cast+transpose block b of hT; DMA to ag_in (if a table is
                    still needed) and accumulate pooling (if layer_out >= 1)."""
                    ptr = pstr.tile([128, 128], dt.bfloat16, tag="tr")
                    nc.tensor.transpose(ptr[:], hT[:, b * BLK:(b + 1) * BLK], ident_sb[:])
                    hrow = epool.tile([128, 128], dt.bfloat16, tag="hrow")
                    nc.scalar.activation(hrow[:], ptr[:], AF.Copy)
                    if layer_out < L:
                        nc.scalar.dma_start(ag_in[b * BLK:(b + 1) * BLK, :], hrow[:])
                    if layer_out >= 1:
                        nc.tensor.matmul(
                            pool_psums[layer_out - 1][:],
                            ohg_sb[:, b, :],
                            hrow[:],
                            start=(b == 0),
                            stop=(b == NBLK - 1),
                            skip_group_check=True,
                        )

                # ---- pre-MLP: hT = relu(relu(x Wp1 + bp1) Wp2 + bp2), transposed;
                # h0 blocks are emitted to ag_in as soon as each chunk lands ----
                o = 0
                while o < PERP:
                    cw = min(MLP_CHUNK, PERP - o)
                    xc = zpool.tile([F, MLP_CHUNK], dt.bfloat16, tag="xc")
                    nc.sync.dma_start(xc[:, :cw], xT_p[:, o:o + cw])
                    p1 = psmlp.tile([F, MLP_CHUNK], dt.float32, tag="mlp")
                    nc.tensor.matmul(p1[:, :cw], Wp1[:], xc[:, :cw])
                    t1 = zpool.tile([F, MLP_CHUNK], dt.bfloat16, tag="t1")
                    nc.scalar.activation(t1[:, :cw], p1[:, :cw], AF.Relu, bias=bp1[:])
                    p2 = psmlp.tile([F, MLP_CHUNK], dt.float32, tag="mlp")
                    nc.tensor.matmul(p2[:, :cw], Wp2[:], t1[:, :cw])
                    nc.scalar.activation(hT[:, o:o + cw], p2[:, :cw], AF.Relu, bias=bp2[:])
                    for k in range(cw // BLK):
                        _emit_block(o // BLK + k, 0)
                    o += cw

                nc.gpsimd.collective_compute(
                    "AllGather", OP.bypass,
                    replica_groups=[list(range(NC))],
                    ins=[ag_in[:]], outs=[tables[0][:]],
                )

                # ---- GIN layers ----
                for l in range(L):
                    pool_psums.append(pspool.tile([G, F], dt.float32, tag="pool", name=f"poolp{l}"))
                    # prescale: hT *= (1 + eps_l)   (table_l already captured h_l)
                    nc.vector.tensor_scalar(
                        hT[:], hT[:], float(1.0 + eps_vals[l]), None, op0=OP.mult
                    )

                    # PSUM accumulation groups are bank-granular: each block gets
                    # its own [F, BLK] psum tile (padded to one bank) and all of
                    # its matmuls are consecutive.  Gathers stay q-major per round
                    # (big calls, queue q -> its own Q7 pair); matmuls consume the
                    # SBUF buffers block-major.
                    K = struct["K"]

                    def _mlp_chunk(o, cw, agg_of):
                        z = zpool.tile([F, MLP_CHUNK], dt.bfloat16, tag="z",
                                       name=f"z_l{l}_o{o}")
                        for k in range(cw // BLK):
                            b = o // BLK + k
                            nc.vector.tensor_tensor(
                                z[:, k * BLK:(k + 1) * BLK],
                                agg_of[b][:],
                                hT[:, b * BLK:(b + 1) * BLK],
                                OP.add,
                            )
                        p1 = psmlp.tile([F, MLP_CHUNK], dt.float32, tag="mlp",
                                        name=f"p1_l{l}_o{o}")
                        nc.tensor.matmul(p1[:, :cw], W1[l][:], z[:, :cw])
                        t1 = zpool.tile([F, MLP_CHUNK], dt.bfloat16, tag="t1",
                                        name=f"t1_l{l}_o{o}")
                        nc.scalar.activation(t1[:, :cw], p1[:, :cw], AF.Relu, bias=b1[l][:])
                        p2 = psmlp.tile([F, MLP_CHUNK], dt.float32, tag="mlp",
                                        name=f"p2_l{l}_o{o}")
                        nc.tensor.matmul(p2[:, :cw], W2[l][:], t1[:, :cw])
                        nc.vector.tensor_scalar(
                            hT[:, o:o + cw], p2[:, :cw], b2[l][:], None, op0=OP.add
                        )
                        for k in range(cw // BLK):
                            _emit_block(o // BLK + k, l + 1)

                    tile_of = struct["tile_of"]
                    vis_done = {b: 0 for b in range(NBLK)}

                    def _issue_gather(g, q, call_off, n_slots, prep):
                        T = n_slots // BLK
                        idxs = ipool.tile([128, n_slots // 16], dt.int16,
                                          tag="idxs", name=f"idxs_l{l}_g{g}_q{q}")
                        nc.sync.dma_start(
                            idxs[:], idx_p[:, call_off // 16:(call_off + n_slots) // 16]
                        )
                        gt = gpool.tile([128, T, 128], dt.bfloat16, tag="gt",
                                        name=f"gt_l{l}_g{g}_q{q}")
                        kw = dict(prepare_only=True, sem=qsems[q]) if prep else {}
                        nc.gpsimd.dma_gather(
                            gt[:],
                            tables[l][q * QS:(q + 1) * QS, :],
                            idxs[:],
                            n_slots,
                            n_slots,
                            F,
                            single_packet=False,
                            queue_num=q,
                            **kw,
                        )
                        return gt, call_off // BLK

                    # g=0 as prepare_only: desc-gen runs on the Q7 pairs while
                    # the table-l AllGather is still in flight; the triggers
                    # fire the transfers the moment the collective lands.
                    gts0 = {}
                    for (gg, q, call_off, n_slots) in call_meta:
                        if gg == 0 and n_slots > 0:
                            gts0[q] = _issue_gather(0, q, call_off, n_slots, False)

                    for g in range(NGR):
                        # issue big gathers (one per quadrant, spanning GRBLK
                        # blocks); queue q -> Q7 pair q so desc-gen pipelines
                        if g == 0:
                            gts = gts0
                        else:
                            gts = {}
                            for (gg, q, call_off, n_slots) in call_meta:
                                if gg == g and n_slots > 0:
                                    gts[q] = _issue_gather(g, q, call_off, n_slots, False)

                        for r in range(g * GRBLK // RBLK,
                                       min((g + 1) * GRBLK, NBLK + RBLK - 1) // RBLK):
                            rblocks = [b for b in _round_blocks(r) if b < NBLK]
                            if not rblocks:
                                continue
                            # load this round's scatter one-hots, per quadrant
                            ohs = {}
                            for q in range(NQ):
                                opos, Tr = oh_off[(r, q)]
                                if Tr == 0 or q not in gts:
                                    continue
                                oh = ohpool.tile([128, Tr, 128], dt.bfloat16, tag="oh",
                                                 name=f"oh_l{l}_r{r}_q{q}")
                                nc.sync.dma_start(
                                    oh[:].rearrange("p a b -> p (a b)"),
                                    ohp_p[:, opos * BLK:(opos + Tr) * BLK],
                                )
                                ohs[q] = (oh, opos)
                            # per-block PSUM accumulators, one full bank each
                            agg_of = {}
                            for b in rblocks:
                                agg_of[b] = psag.tile([F, BLK], dt.float32, tag="agg",
                                                      name=f"agg_l{l}_b{b}")
                                if int(K[:, b].sum()) == 0:
                                    nc.vector.memset(agg_of[b][:], 0.0)
                            for q in range(NQ):
                                if q not in ohs:
                                    continue
                                oh, opos = ohs[q]
                                gt, c0 = gts[q]
                                ot = 0
                                for b in rblocks:
                                    nvis = int(K[:, b].sum())
                                    bt = tile_of[(q, b)]
                                    for t in range(int(K[q, b])):
                                        nc.tensor.matmul(
                                            agg_of[b][:],
                                            gt[:, bt - c0 + t, :],
                                            oh[:, ot + t, :],
                                            start=(vis_done[b] == 0),
                                            stop=(vis_done[b] == nvis - 1),
                                            skip_group_check=True,
                                        )
                                        vis_done[b] += 1
                                    ot += int(K[q, b])
                            # close the round: z, MLP, emit (one chunk per round)
                            o = rblocks[0] * BLK
                            _mlp_chunk(o, (rblocks[-1] + 1) * BLK - o, agg_of)

                    if l + 1 < L:
                        nc.gpsimd.collective_compute(
                            "AllGather", OP.bypass,
                            replica_groups=[list(range(NC))],
                            ins=[ag_in[:]], outs=[tables[l + 1][:]],
                        )
                    # extract pooled sums for this layer
                    pooled_sb = epool.tile([G, F], dt.float32, tag="pooled")
                    nc.scalar.activation(pooled_sb[:], pool_psums[l][:], AF.Copy)
                    nc.scalar.dma_start(out_p[:, l * F:(l + 1) * F], pooled_sb[:])

    nc.compile()
    return nc


def _make_in_maps(struct, inputs):
    x = np.asarray(inputs["x"], dtype=_F32)
    ident = np.eye(128, dtype=_F32).astype(_BF16)

    shared = {
        "ident": np.ascontiguousarray(ident),
        "Wp1": np.asarray(inputs["W_pre1"], dtype=_F32).astype(_BF16),
        "bp1": np.asarray(inputs["b_pre1"], dtype=_F32).reshape(F, 1),
        "Wp2": np.asarray(inputs["W_pre2"], dtype=_F32).astype(_BF16),
        "bp2": np.asarray(inputs["b_pre2"], dtype=_F32).reshape(F, 1),
        "W1s": np.asarray(inputs["W1s"], dtype=_F32).astype(_BF16),
        "b1s": np.asarray(inputs["b1s"], dtype=_F32).reshape(L, F, 1),
        "W2s": np.asarray(inputs["W2s"], dtype=_F32).astype(_BF16),
        "b2s": np.asarray(inputs["b2s"], dtype=_F32).reshape(L, F, 1),
    }

    perm = struct["perm"]
    in_maps = []
    for c in range(NC):
        xs = np.zeros((F, PERP), dtype=_F32)
        occ = perm[c] >= 0
        xs[:, occ] = x[perm[c][occ]].T
        xs = xs.astype(_BF16)
        m = dict(shared)
        m["xT"] = xs
        m["idx"] = struct["idx_planes"][c]
        m["ohp"] = struct["oh_planes"][c]
        m["ohg"] = struct["ohg_planes"][c]
        in_maps.append(m)
    return in_maps


def kernel(**inputs):
    from concourse.bass_utils import run_bass_kernel_spmd

    edge_index = np.asarray(inputs["edge_index"])
    batch = np.asarray(inputs["batch"])
    eps = np.asarray(inputs["eps"], dtype=_F32)

    struct = _build_structures(edge_index, batch)
    nc = _build_program(struct, [float(e) for e in eps])
    in_maps = _make_in_maps(struct, inputs)

    res = run_bass_kernel_spmd(nc, in_maps, core_ids=list(range(NC)))
    out = np.zeros((G, L * F), dtype=_F32)
    for c in range(NC):
        out += res.results[c]["out"]
    return out
